# revision 1
# baseline (speedup 1.0000x reference)
"""Trainium2 Bass kernel for DinoVisionTransformer Sparse-MoE FC2 (LoRA experts).

Computation (per token t):
    logits = x @ Wg                      -> top-2 softmax-renormalized weights
    out    = x @ W2 + b2 + sum_e cw[t,e] * scale[e] * (x @ A_e) @ B_e

Sharding: data-parallel over the batch dim (8 batch rows -> 8 NeuronCores,
1024 tokens each). All weights replicated.

Per-core kernel (fp16 compute, fp32 PSUM accumulation):
  Phase A (per 128-token tile, contraction over H=4096 in 32 k-chunks of 128):
    stationary = xT tile [128h x 128t]; moving = Wcat[k] columns where
    Wcat = [W2 (1024) | A_flat (512) | Wg_hi (8) | Wg_lo (8)]  (fp16)
    -> psum_base [128,1024], psum_h [128,512], psum_L [128,16]
    Router logits get near-fp32 precision via the split x = x_hi + x_lo:
    logits = x_hi@Wg_hi + x_hi@Wg_lo + x_lo@Wg_hi  (all accumulated in fp32)
  Router (DVE): top-2 of 8 via max8; w1 = sigmoid(l1-l2), w2 = 1-w1;
    dense combine weights cw[t,e] by equality masks. scale[] folded into Bm.
  LoRA: h weighted by cw, transposed 128x128 via PE, then phase B matmuls
    (contraction over E*R=512) accumulate directly into psum_base.
  Final: out = psum_base + b2 (broadcast), DMA to DRAM.
"""

import sys

if "/opt/trn_rl_repo" not in sys.path:
    sys.path.insert(0, "/opt/trn_rl_repo")

import numpy as np

import concourse.bass as bass  # noqa: F401  (registers types)
import concourse.mybir as mybir
import concourse.tile as tile
from concourse import bacc
from concourse.bass import ts
from concourse.bass_utils import run_bass_kernel_spmd
from concourse.masks import make_identity

P = 128
KCH = 32          # H / 128 contraction chunks
TT = 8            # 128-token tiles per core
H = 4096
D = 1024
E = 8
R = 64
ER = E * R        # 512
NW = D + ER + 8 + 8   # 1552 wcat columns
NCORES = 8
WG_K_GROUPS = 8   # wcat DMA split granularity (k-chunks per group)
KPG = KCH // WG_K_GROUPS  # 4

F16 = mybir.dt.float16
F32 = mybir.dt.float32

_CACHE = {}


def _build_nc():
    nc = bacc.Bacc("TRN2")

    xtb_d = nc.dram_tensor("xtb", [TT, P, KCH, P], F16, kind="ExternalInput")
    xlo_d = nc.dram_tensor("xlo", [TT, P, KCH, P], F16, kind="ExternalInput")
    wcat_d = nc.dram_tensor("wcat", [P, KCH, NW], F16, kind="ExternalInput")
    bm_d = nc.dram_tensor("bm", [P, 4, D], F16, kind="ExternalInput")
    b2b_d = nc.dram_tensor("b2b", [P, D], F32, kind="ExternalInput")
    y_d = nc.dram_tensor("y", [TT * P, D], F32, kind="ExternalOutput")

    Sig = mybir.ActivationFunctionType.Sigmoid
    Alu = mybir.AluOpType

    with tile.TileContext(nc) as tc:
        with (
            tc.tile_pool(name="wres", bufs=1) as wres,
            tc.tile_pool(name="xin", bufs=3) as xin,
            tc.tile_pool(name="small", bufs=2) as small,
            tc.tile_pool(name="hbuf", bufs=2) as hbuf,
            tc.tile_pool(name="obuf", bufs=2) as obuf,
            tc.tile_pool(name="ps_base", bufs=2, space="PSUM") as ps_base_pool,
            tc.tile_pool(name="ps_h", bufs=2, space="PSUM") as ps_h_pool,
            tc.tile_pool(name="ps_l", bufs=1, space="PSUM") as ps_l_pool,
            tc.tile_pool(name="ps_t", bufs=1, space="PSUM") as ps_t_pool,
        ):
            # ---- x_hi tiles 0,1 first so phase A can start early; xlo
            # tiles are only needed by the delayed router-correction matmuls,
            # so they issue after the first wcat chunks. HWDGE drains roughly
            # in issue order -> issue in consumption order, no dep chains. ----
            xts = {}
            xlo01 = {}
            for t0 in (0, 1):
                xtb_ = xin.tile([P, KCH, P], F16, tag="xtb")
                xlo_ = xin.tile([P, KCH, P], F16, tag="xlo")
                nc.sync.dma_start(xtb_[:], xtb_d[t0])
                xlo01[t0] = xlo_
                xts[t0] = (xtb_, xlo_)
            wcat_sb = []
            for g in range(WG_K_GROUPS):
                t_ = wres.tile([P, KPG, NW], F16, tag=f"wcat{g}")
                nc.sync.dma_start(t_[:], wcat_d[:, ts(g, KPG), :])
                wcat_sb.append(t_)
                if g == 3:
                    nc.sync.dma_start(xlo01[0][:], xlo_d[0])
                    nc.sync.dma_start(xlo01[1][:], xlo_d[1])
            bm_sb = wres.tile([P, 4, D], F16, tag="bm")
            nc.sync.dma_start(bm_sb[:], bm_d[:])
            b2b_sb = wres.tile([P, D], F32, tag="b2b")
            nc.sync.dma_start(b2b_sb[:], b2b_d[:])
            for t0 in (2, 3):
                xtb_ = xin.tile([P, KCH, P], F16, tag="xtb")
                xlo_ = xin.tile([P, KCH, P], F16, tag="xlo")
                nc.sync.dma_start(xtb_[:], xtb_d[t0])
                nc.sync.dma_start(xlo_[:], xlo_d[t0])
                xts[t0] = (xtb_, xlo_)
            ident = wres.tile([P, P], F16, tag="ident")
            make_identity(nc, ident[:])

            def wc(k, lo, hi):
                return wcat_sb[k // KPG][:, k % KPG, lo:hi]

            # shared logits psum bank: tile t uses half (t % 2).
            # cols [0:16] = x_hi @ [Wg_hi | Wg_lo]; cols [16:24] = 1024x the
            # xlo correction (fp8 operands; rescaled on the DVE afterwards)
            ps_l_shared = ps_l_pool.tile([P, 64], F32, tag="l")

            pend = {}   # t -> (ps_base, ps_h, hwT or None)

            def emit_A_group(t, g, late_xlo=False, warm_only=False):
                """Phase-A matmuls for k-chunks [g*KPG, (g+1)*KPG) of tile t.

                late_xlo: bunch the xlo-correction matmuls into the second
                half of the k-loop (two per slot) so the xlo DMA can be
                issued after the first wcat chunks during startup."""
                xtb_sb, xlo_sb = xts[t]
                if isinstance(xtb_sb, list):
                    def xap(k, _x=xtb_sb):
                        return _x[k // 8][:, k % 8, :]
                else:
                    def xap(k, _x=xtb_sb):
                        return _x[:, k, :]
                ps_base, ps_h, _ = pend[t]
                ps_l = ps_l_shared[:, (t % 2) * 32:(t % 2) * 32 + 32]
                for k in range(g * KPG, (g + 1) * KPG):
                    st = k == 0
                    fin = k == KCH - 1
                    # order: tiny-N matmuls sit between 512-col streams so
                    # their self-loading weight fetches hide under the streams
                    nc.tensor.matmul(
                        ps_base[:, 0:512], xap(k), wc(k, 0, 512),
                        start=st, stop=False, skip_group_check=True,
                    )
                    if not warm_only:
                        nc.tensor.matmul(
                            ps_l[:, 0:16], xap(k), wc(k, 1536, 1552),
                            start=False, stop=False, skip_group_check=True,
                        )
                    nc.tensor.matmul(
                        ps_base[:, 512:1024], xap(k), wc(k, 512, 1024),
                        start=st, stop=False, skip_group_check=True,
                    )
                    nc.tensor.matmul(
                        ps_h[:, :], xap(k), wc(k, 1024, 1536),
                        start=st, stop=fin, skip_group_check=True,
                    )
                    if warm_only:
                        continue
                    if late_xlo:
                        if k >= KCH // 2:
                            for kx in (k - KCH // 2, k):
                                nc.tensor.matmul(
                                    ps_l[:, 0:8], xlo_sb[:, kx, :],
                                    wc(kx, 1536, 1544),
                                    start=False, stop=(kx == KCH - 1),
                                    skip_group_check=True,
                                )
                    else:
                        nc.tensor.matmul(
                            ps_l[:, 0:8], xlo_sb[:, k, :], wc(k, 1536, 1544),
                            start=False, stop=fin, skip_group_check=True,
                        )

            def emit_router_dve(t):
                """Router math + h-weighting (DVE/ACT only); returns hw."""
                ps_base, ps_h, _ = pend[t]
                ps_l = ps_l_shared[:, (t % 2) * 32:(t % 2) * 32 + 32]
                logits = small.tile([P, 8], F32, tag="logits")
                nc.vector.tensor_reduce(
                    logits[:],
                    ps_l[:, 0:16].rearrange("p (s j) -> p j s", s=2),
                    axis=mybir.AxisListType.X,
                    op=Alu.add,
                )
                m8 = small.tile([P, 8], F32, tag="m8")
                nc.vector.max(m8[:], logits[:])
                g_ = small.tile([P, 1], F32, tag="gap")
                nc.vector.tensor_sub(g_[:], m8[:, 0:1], m8[:, 1:2])
                w1 = small.tile([P, 1], F32, tag="w1")
                nc.scalar.activation(w1[:], g_[:], Sig)
                w2 = small.tile([P, 1], F32, tag="w2")
                nc.scalar.activation(w2[:], g_[:], Sig, scale=-1.0)
                cw = small.tile([P, 8], F32, tag="cw")
                cwb = small.tile([P, 8], F32, tag="cwb")
                nc.vector.scalar_tensor_tensor(
                    cw[:], logits[:], m8[:, 0:1], w1[:, 0:1].to_broadcast([P, 8]),
                    op0=Alu.is_equal, op1=Alu.mult,
                )
                nc.vector.scalar_tensor_tensor(
                    cwb[:], logits[:], m8[:, 1:2], w2[:, 0:1].to_broadcast([P, 8]),
                    op0=Alu.is_equal, op1=Alu.mult,
                )
                nc.vector.tensor_add(cw[:], cw[:], cwb[:])
                hw = hbuf.tile([P, ER], F16, tag="hw")
                nc.vector.tensor_tensor(
                    hw.rearrange("p (e r) -> p e r", e=E),
                    ps_h.rearrange("p (e r) -> p e r", e=E),
                    cw[:, :, None].to_broadcast([P, E, R]),
                    Alu.mult,
                )
                return hw

            def emit_router_pe(t, hw):
                """PE transposes of weighted h + copy back; fills pend[t] hwT."""
                ps_base, ps_h, _ = pend[t]
                ps_t = ps_t_pool.tile([P, ER], F16, tag="t")
                for j in range(4):
                    nc.tensor.transpose(
                        ps_t[:, ts(j, P)], hw[:, ts(j, P)], ident[:]
                    )
                hwT = hbuf.tile([P, 4, P], F16, tag="hwT")
                nc.vector.tensor_copy(hwT.rearrange("p a b -> p (a b)"), ps_t[:])
                pend[t] = (ps_base, ps_h, hwT)

            def emit_router(t):
                emit_router_pe(t, emit_router_dve(t))

            def emit_B_and_out(t):
                """LoRA phase B accumulated into base psum, bias add, store."""
                ps_base, _, hwT = pend.pop(t)
                for j in range(4):
                    nc.tensor.matmul(
                        ps_base[:, 0:512], hwT[:, j, :], bm_sb[:, j, 0:512],
                        start=False, stop=False, skip_group_check=True,
                    )
                    nc.tensor.matmul(
                        ps_base[:, 512:1024], hwT[:, j, :], bm_sb[:, j, 512:1024],
                        start=False, stop=(j == 3), skip_group_check=True,
                    )
                out_sb = obuf.tile([P, D], F32, tag="out")
                nc.vector.tensor_add(out_sb[:], ps_base[:], b2b_sb[:])
                nc.scalar.dma_start(y_d[ts(t, P), :], out_sb[:])

            def alloc_psums(t):
                pend[t] = (
                    ps_base_pool.tile([P, D], F32, tag="base", name=f"base{t}"),
                    ps_h_pool.tile([P, ER], F32, tag="h", name=f"h{t}"),
                    None,
                )
                # The shared logits bank must never see start=True (a bank-wide
                # has_written clear would wipe the other tile's half). Instead
                # zero this tile's half; start=False matmuls then accumulate
                # onto 0 (bits set) or overwrite with v (bits clear) — both ok.
                nc.vector.memset(
                    ps_l_shared[:, (t % 2) * 32:(t % 2) * 32 + 32], 0.0
                )

            # ---- startup: interleave phase A of tiles 0 and 1 so the PE has
            # two tiles of work while wcat chunks stream in ----
            D_OFF = 2
            alloc_psums(0)
            alloc_psums(1)
            for g in range(WG_K_GROUPS + D_OFF):
                if g < WG_K_GROUPS:
                    emit_A_group(0, g, late_xlo=True)
                if g == WG_K_GROUPS:
                    emit_router(0)
                gg = g - D_OFF
                if 0 <= gg < WG_K_GROUPS:
                    emit_A_group(1, gg, late_xlo=True)
                if gg == WG_K_GROUPS - 2:
                    emit_B_and_out(0)
            hw_pend = {1: emit_router_dve(1)}

            # ---- steady state ----
            for t in range(2, TT - 1):
                if t >= 4:
                    xtb_ = xin.tile([P, KCH, P], F16, tag="xtb")
                    xlo_ = xin.tile([P, KCH, P], F16, tag="xlo")
                    nc.sync.dma_start(xtb_[:], xtb_d[t])
                    nc.sync.dma_start(xlo_[:], xlo_d[t])
                    xts[t] = (xtb_, xlo_)
                alloc_psums(t)
                for g in range(WG_K_GROUPS):
                    emit_A_group(t, g)
                    if g == 0 and (t - 1) in hw_pend:
                        # previous tile's transposes here: its DVE router
                        # chain is long done, so the PE never stalls on it
                        emit_router_pe(t - 1, hw_pend.pop(t - 1))
                    if g == 4:
                        # previous tile's phase B mid-A so its psum/base slot
                        # frees well before tile t+1 needs it
                        emit_B_and_out(t - 1)
                hw_pend[t] = emit_router_dve(t)

            # ---- last tile: router columns (L, h) stream first so the DVE
            # router chain overlaps the base-column streams; transposes are
            # injected mid-loop -> phase B follows the final matmul directly
            t = TT - 1
            xtb_ = xin.tile([P, KCH, P], F16, tag="xtb")
            xlo_ = xin.tile([P, KCH, P], F16, tag="xlo")
            nc.sync.dma_start(xtb_[:], xtb_d[t])
            nc.sync.dma_start(xlo_[:], xlo_d[t])
            xts[t] = (xtb_, xlo_)
            alloc_psums(t)
            xtb_sb, xlo_sb = xts[t]
            ps_base, ps_h, _ = pend[t]
            ps_l = ps_l_shared[:, (t % 2) * 32:(t % 2) * 32 + 32]
            for k in range(KCH):
                st = k == 0
                fin = k == KCH - 1
                nc.tensor.matmul(
                    ps_l[:, 0:16], xtb_sb[:, k, :], wc(k, 1536, 1552),
                    start=False, stop=False, skip_group_check=True,
                )
                if k == 4 and (t - 1) in hw_pend:
                    emit_router_pe(t - 1, hw_pend.pop(t - 1))
                nc.tensor.matmul(
                    ps_h[:, :], xtb_sb[:, k, :], wc(k, 1024, 1536),
                    start=st, stop=fin, skip_group_check=True,
                )
                nc.tensor.matmul(
                    ps_l[:, 0:8], xlo_sb[:, k, :], wc(k, 1536, 1544),
                    start=False, stop=fin, skip_group_check=True,
                )
            hw_last = emit_router_dve(t)
            for k in range(KCH):
                st = k == 0
                nc.tensor.matmul(
                    ps_base[:, 0:512], xtb_sb[:, k, :], wc(k, 0, 512),
                    start=st, stop=False, skip_group_check=True,
                )
                nc.tensor.matmul(
                    ps_base[:, 512:1024], xtb_sb[:, k, :], wc(k, 512, 1024),
                    start=st, stop=(k == KCH - 1), skip_group_check=True,
                )
                if k == 4:
                    emit_B_and_out(t - 1)
                if k == 10:
                    emit_router_pe(t, hw_last)
            # drain: last tile's phase B with split evac so the first half's
            # bias-add + store overlap the second half's matmuls
            ps_base, _, hwT = pend.pop(TT - 1)
            for j in range(4):
                nc.tensor.matmul(
                    ps_base[:, 0:512], hwT[:, j, :], bm_sb[:, j, 0:512],
                    start=False, stop=(j == 3), skip_group_check=True,
                )
            out_sb = obuf.tile([P, D], F32, tag="out")
            nc.vector.tensor_add(
                out_sb[:, 0:512], ps_base[:, 0:512], b2b_sb[:, 0:512]
            )
            nc.scalar.dma_start(
                y_d[ts(TT - 1, P), 0:512], out_sb[:, 0:512]
            )
            for j in range(4):
                nc.tensor.matmul(
                    ps_base[:, 512:1024], hwT[:, j, :], bm_sb[:, j, 512:1024],
                    start=False, stop=(j == 3), skip_group_check=True,
                )
            nc.vector.tensor_add(
                out_sb[:, 512:1024], ps_base[:, 512:1024], b2b_sb[:, 512:1024]
            )
            nc.scalar.dma_start(
                y_d[ts(TT - 1, P), 512:1024], out_sb[:, 512:1024]
            )

    nc.finalize()
    return nc


def _prep_shared(Wg, W2, b2, A, Bm, scale):
    """Host-side weight layout prep (replicated across cores)."""
    f16, f32 = np.float16, np.float32
    # Wcat = [W2 | A_flat | Wg_hi | Wg_lo], k-chunked to [128, 32, NW]
    a_flat = np.ascontiguousarray(A.transpose(1, 0, 2)).reshape(H, ER)
    wg_hi = Wg.astype(f16)
    wg_lo = (Wg.astype(f32) - wg_hi.astype(f32)).astype(f16)
    wcat = np.empty((H, NW), dtype=f16)
    wcat[:, 0:D] = W2.astype(f16)
    wcat[:, D:D + ER] = a_flat.astype(f16)
    wcat[:, D + ER:D + ER + 8] = wg_hi
    wcat[:, D + ER + 8:] = wg_lo
    wcat = np.ascontiguousarray(wcat.reshape(KCH, P, NW).transpose(1, 0, 2))

    # Bm with scale folded, [(e r), d] -> [128, 4, D]
    bms = (Bm.astype(f32) * scale.astype(f32)[:, None, None]).reshape(ER, D)
    bms = np.ascontiguousarray(bms.reshape(4, P, D).transpose(1, 0, 2)).astype(f16)

    b2b = np.ascontiguousarray(
        np.broadcast_to(b2.astype(f32)[None, :], (P, D))
    )
    return wcat, bms, b2b


def _prep_x_core(x_c):
    """Per-core x prep: fp16 hi + scaled-fp8 lo split, [tile, p, k, ti] layout."""
    f16, f32 = np.float16, np.float32
    xtb = x_c.astype(f16)                                   # [1024, 4096]
    xlo = (x_c.astype(f32) - xtb.astype(f32)).astype(f16)
    def lay(a):
        return np.ascontiguousarray(
            a.reshape(TT, P, KCH, P).transpose(0, 3, 2, 1)
        )
    return lay(xtb), lay(xlo)


def kernel(x, Wg, W2, b2, A, Bm, scale):
    x = np.asarray(x, dtype=np.float32)
    Wg = np.asarray(Wg, dtype=np.float32)
    W2 = np.asarray(W2, dtype=np.float32)
    b2 = np.asarray(b2, dtype=np.float32)
    A = np.asarray(A, dtype=np.float32)
    Bm = np.asarray(Bm, dtype=np.float32)
    scale = np.asarray(scale, dtype=np.float32)

    if "nc" not in _CACHE:
        _CACHE["nc"] = _build_nc()
    nc = _CACHE["nc"]

    wcat, bms, b2b = _prep_shared(Wg, W2, b2, A, Bm, scale)
    in_maps = []
    for c in range(NCORES):
        xtb, xlo = _prep_x_core(x[c])
        in_maps.append(
            {"xtb": xtb, "xlo": xlo, "wcat": wcat, "bm": bms, "b2b": b2b}
        )

    res = run_bass_kernel_spmd(nc, in_maps, core_ids=list(range(NCORES)))
    out = np.stack([res.results[c]["y"] for c in range(NCORES)], axis=0)
    return out.astype(np.float32)



# revision 6
# speedup vs baseline: 1.0895x; 1.0895x over previous
"""Trainium2 Bass kernel for DinoVisionTransformer Sparse-MoE FC2 (LoRA experts).

Computation (per token t):
    logits = x @ Wg                      -> top-2 softmax-renormalized weights
    out    = x @ W2 + b2 + sum_e cw[t,e] * scale[e] * (x @ A_e) @ B_e

Sharding: data-parallel over the batch dim (8 batch rows -> 8 NeuronCores,
1024 tokens each). All weights replicated.

Per-core kernel (mixed fp16 / fp8e4m3 compute, fp32 PSUM accumulation):
  The base FC2 (x @ W2) runs in fp16 (precision-critical: fp8 base alone
  gives ~3.6e-2 rel err, over the 2e-2 gate). The LoRA expert path (phase A
  x @ A_flat, phase B h @ Bm) and the router xlo correction run in fp8e4
  with DoubleRow perf mode (2 k-chunks per instruction, 2x PE rate);
  measured end-to-end error of this split is ~7e-3.

  Scaling scheme (all power-of-2, exact): W2, A, Bm are uploaded x64 so
  fp8/fp16 mantissas sit in the normal range (A,Bm entries ~0.02 std would
  otherwise hit e4m3 subnormals). ps_base accumulates 64*(x@W2); phase A
  produces ps_h = 64*h; the DVE combine weight is cw*scale/64 so hw = h*cw*
  scale; phase B adds hw@(64*Bm) = 64*delta into ps_base; final evac does
  out = ps_base/64 + b2 in one scalar_tensor_tensor.

  Router logits keep near-fp32 precision: logits = xtb@Wg_hi + xtb@Wg_lo
  (fp16) + 2^-17 * (xlo8 @ Wg8) where xlo8 = fp8((x - fp16(x)) * 2^11),
  Wg8 = fp8(Wg * 64). Top-2 of 8 via max8; w1 = sigmoid(l1-l2), w2 = 1-w1;
  dense combine weights by equality masks, then * scale/64.

  Per 128-token tile: phase A first (16 DoubleRow matmuls), then the base
  k-loop streams wcat columns [W2*64 | Wg_hi | Wg_lo], then the xlo
  correction (16 narrow DoubleRow), router DVE chain, PE transposes of the
  weighted h (fp16), fp8 cast on copy-back, phase B accumulated into the
  base psum, bias add + store.
"""

import sys

if "/opt/trn_rl_repo" not in sys.path:
    sys.path.insert(0, "/opt/trn_rl_repo")

import ml_dtypes
import numpy as np

import concourse.bass as bass  # noqa: F401  (registers types)
import concourse.mybir as mybir
import concourse.tile as tile
from concourse import bacc
from concourse.bass import ts
from concourse.bass_utils import run_bass_kernel_spmd
from concourse.masks import make_identity

P = 128
KCH = 32          # H / 128 contraction chunks
TT = 8            # 128-token tiles per core
H = 4096
D = 1024
E = 8
R = 64
ER = E * R        # 512
NW16 = D + 16     # wcat16 columns: W2*64 | Wg_hi | Wg_lo
NW8 = ER + 8      # acat8 columns: A_flat*64 | Wg8
NCORES = 8
WG_K_GROUPS = 8   # wcat16 DMA split granularity (k-chunks per group)
KPG = KCH // WG_K_GROUPS  # 4
ACH = 4           # acat8 DMA chunks (8 k-chunks each)

F16 = mybir.dt.float16
F32 = mybir.dt.float32
F8 = mybir.dt.float8e4
DR = mybir.MatmulPerfMode.DoubleRow

_CACHE = {}


def _build_nc():
    nc = bacc.Bacc("TRN2")

    xtb_d = nc.dram_tensor("xtb", [TT, P, KCH, P], F16, kind="ExternalInput")
    x8_d = nc.dram_tensor("x8", [TT, P, KCH, P], F8, kind="ExternalInput")
    xlo_d = nc.dram_tensor("xlo", [TT, P, KCH, P], F8, kind="ExternalInput")
    wcat_d = nc.dram_tensor("wcat", [P, KCH, NW16], F16, kind="ExternalInput")
    acat_d = nc.dram_tensor("acat", [P, KCH, NW8], F8, kind="ExternalInput")
    bm_d = nc.dram_tensor("bm", [P, 4, D], F8, kind="ExternalInput")
    b2b_d = nc.dram_tensor("b2b", [P, D], F32, kind="ExternalInput")
    sc_d = nc.dram_tensor("sc", [P, E], F32, kind="ExternalInput")
    y_d = nc.dram_tensor("y", [TT * P, D], F32, kind="ExternalOutput")

    Sig = mybir.ActivationFunctionType.Sigmoid
    Alu = mybir.AluOpType

    with tile.TileContext(nc) as tc:
        with (
            tc.tile_pool(name="wres", bufs=1) as wres,
            tc.tile_pool(name="xin", bufs=3) as xin,
            tc.tile_pool(name="small", bufs=2) as small,
            tc.tile_pool(name="hbuf", bufs=2) as hbuf,
            tc.tile_pool(name="obuf", bufs=2) as obuf,
            tc.tile_pool(name="ps_base", bufs=2, space="PSUM") as ps_base_pool,
            tc.tile_pool(name="ps_h", bufs=2, space="PSUM") as ps_h_pool,
            tc.tile_pool(name="ps_l", bufs=1, space="PSUM") as ps_l_pool,
            tc.tile_pool(name="ps_t", bufs=1, space="PSUM") as ps_t_pool,
        ):
            # ---- DMA issue order = consumption order (HWDGE drains roughly
            # in issue order). Phase A of tile 0 only needs x8_0 + the first
            # acat chunks, so the PE can start ~6us before wcat g0 lands. ----
            xts = {}

            def alloc_x(t):
                xts[t] = (
                    xin.tile([P, KCH, P], F16, tag="xtb", name=f"xtb{t}"),
                    xin.tile([P, KCH, P], F8, tag="x8", name=f"x8_{t}"),
                    xin.tile([P, KCH, P], F8, tag="xlo", name=f"xlo{t}"),
                )

            def dma_x(t, which="all"):
                xtb_, x8_, xlo_ = xts[t]
                if which in ("all", "x8"):
                    nc.sync.dma_start(x8_[:], x8_d[t])
                if which in ("all", "xtb"):
                    nc.sync.dma_start(xtb_[:], xtb_d[t])
                if which in ("all", "xlo"):
                    nc.sync.dma_start(xlo_[:], xlo_d[t])

            alloc_x(0)
            alloc_x(1)
            dma_x(0, "x8")
            acat_sb = []
            for c in range(ACH):
                acat_sb.append(
                    wres.tile([P, KCH // ACH, NW8], F8, tag=f"acat{c}",
                              name=f"acat{c}")
                )
            nc.sync.dma_start(acat_sb[0][:], acat_d[:, ts(0, KCH // ACH), :])
            nc.sync.dma_start(acat_sb[1][:], acat_d[:, ts(1, KCH // ACH), :])
            dma_x(0, "xtb")
            wcat_sb = []
            for g in range(WG_K_GROUPS):
                wcat_sb.append(
                    wres.tile([P, KPG, NW16], F16, tag=f"wcat{g}",
                              name=f"wcat{g}")
                )
            nc.sync.dma_start(wcat_sb[0][:], wcat_d[:, ts(0, KPG), :])
            nc.sync.dma_start(acat_sb[2][:], acat_d[:, ts(2, KCH // ACH), :])
            nc.sync.dma_start(acat_sb[3][:], acat_d[:, ts(3, KCH // ACH), :])
            dma_x(1, "x8")
            nc.sync.dma_start(wcat_sb[1][:], wcat_d[:, ts(1, KPG), :])
            dma_x(1, "xtb")
            bm_sb = wres.tile([P, 4, D], F8, tag="bm")
            b2b_sb = wres.tile([P, D], F32, tag="b2b")
            sc_sb = wres.tile([P, E], F32, tag="sc")
            for g in range(2, WG_K_GROUPS):
                nc.sync.dma_start(wcat_sb[g][:], wcat_d[:, ts(g, KPG), :])
                if g == 3:
                    dma_x(0, "xlo")
                elif g == 4:
                    dma_x(1, "xlo")
                elif g == 5:
                    nc.sync.dma_start(bm_sb[:], bm_d[:])
                    nc.sync.dma_start(b2b_sb[:], b2b_d[:])
                    nc.sync.dma_start(sc_sb[:], sc_d[:])
                elif g == 6:
                    alloc_x(2)
                    dma_x(2)
                elif g == 7:
                    alloc_x(3)
                    dma_x(3)
            ident = wres.tile([P, P], F16, tag="ident")
            make_identity(nc, ident[:])

            def wc(k, lo, hi):
                return wcat_sb[k // KPG][:, k % KPG, lo:hi]

            def ac(kp, lo, hi):
                # k-pair kp covers k-chunks 2kp, 2kp+1; acat chunk c = kp//4
                c = kp // 4
                j = 2 * (kp % 4)
                return acat_sb[c][:, j:j + 2, lo:hi]

            # shared logits psum bank: tile t uses half (t % 2).
            # cols [0:16] = xtb @ [Wg_hi | Wg_lo]; cols [16:24] = 2^17x the
            # xlo correction (fp8 operands; rescaled on the DVE afterwards)
            ps_l_shared = ps_l_pool.tile([P, 64], F32, tag="l")

            pend = {}   # t -> (ps_base, ps_h, hwT or None)

            def lhalf(t):
                o = (t % 2) * 32
                return ps_l_shared[:, o:o + 32]

            def alloc_psums(t):
                pend[t] = (
                    ps_base_pool.tile([P, D], F32, tag="base", name=f"base{t}"),
                    ps_h_pool.tile([P, ER], F32, tag="h", name=f"h{t}"),
                    None,
                )
                # The shared logits bank must never see start=True (a bank-wide
                # has_written clear would wipe the other tile's half). Instead
                # zero this tile's half; start=False matmuls then accumulate
                # onto 0 (bits set) or overwrite with v (bits clear) — both ok.
                nc.vector.memset(lhalf(t)[:, 0:24], 0.0)

            def emit_A(t):
                """Phase A: ps_h = x8 @ A8 (fp8 DoubleRow, 2 k-chunks/instr)."""
                _, x8_sb, _ = xts[t]
                _, ps_h, _ = pend[t]
                for kp in range(KCH // 2):
                    nc.tensor.matmul(
                        ps_h[:, :], x8_sb[:, 2 * kp:2 * kp + 2, :],
                        ac(kp, 0, ER),
                        start=(kp == 0), stop=(kp == KCH // 2 - 1),
                        perf_mode=DR, skip_group_check=True,
                    )

            def emit_xlo(t):
                """Router xlo correction into ps_l[16:24] (fp8 DoubleRow)."""
                _, _, xlo_sb = xts[t]
                ps_l = lhalf(t)
                for kp in range(KCH // 2):
                    nc.tensor.matmul(
                        ps_l[:, 16:24], xlo_sb[:, 2 * kp:2 * kp + 2, :],
                        ac(kp, ER, ER + 8),
                        start=False, stop=(kp == KCH // 2 - 1),
                        perf_mode=DR, skip_group_check=True,
                    )

            def emit_base_group(t, g):
                """Base FC2 + router-hi matmuls for k-chunks of group g."""
                xtb_sb, _, _ = xts[t]
                ps_base, _, _ = pend[t]
                ps_l = lhalf(t)
                for k in range(g * KPG, (g + 1) * KPG):
                    st = k == 0
                    nc.tensor.matmul(
                        ps_base[:, 0:512], xtb_sb[:, k, :], wc(k, 0, 512),
                        start=st, stop=False, skip_group_check=True,
                    )
                    nc.tensor.matmul(
                        ps_l[:, 0:16], xtb_sb[:, k, :], wc(k, D, NW16),
                        start=False, stop=(k == KCH - 1),
                        skip_group_check=True,
                    )
                    nc.tensor.matmul(
                        ps_base[:, 512:1024], xtb_sb[:, k, :], wc(k, 512, 1024),
                        start=st, stop=False, skip_group_check=True,
                    )

            def emit_router_dve(t):
                """Router math + h-weighting (DVE/ACT only); returns hw."""
                _, ps_h, _ = pend[t]
                ps_l = lhalf(t)
                logits0 = small.tile([P, 8], F32, tag="logits0")
                nc.vector.tensor_reduce(
                    logits0[:],
                    ps_l[:, 0:16].rearrange("p (s j) -> p j s", s=2),
                    axis=mybir.AxisListType.X,
                    op=Alu.add,
                )
                logits = small.tile([P, 8], F32, tag="logits")
                nc.vector.scalar_tensor_tensor(
                    logits[:], ps_l[:, 16:24], 2.0 ** -17, logits0[:],
                    op0=Alu.mult, op1=Alu.add,
                )
                m8 = small.tile([P, 8], F32, tag="m8")
                nc.vector.max(m8[:], logits[:])
                g_ = small.tile([P, 1], F32, tag="gap")
                nc.vector.tensor_sub(g_[:], m8[:, 0:1], m8[:, 1:2])
                w1 = small.tile([P, 1], F32, tag="w1")
                nc.scalar.activation(w1[:], g_[:], Sig)
                w2 = small.tile([P, 1], F32, tag="w2")
                nc.scalar.activation(w2[:], g_[:], Sig, scale=-1.0)
                cw = small.tile([P, 8], F32, tag="cw")
                cwb = small.tile([P, 8], F32, tag="cwb")
                nc.vector.scalar_tensor_tensor(
                    cw[:], logits[:], m8[:, 0:1], w1[:, 0:1].to_broadcast([P, 8]),
                    op0=Alu.is_equal, op1=Alu.mult,
                )
                nc.vector.scalar_tensor_tensor(
                    cwb[:], logits[:], m8[:, 1:2], w2[:, 0:1].to_broadcast([P, 8]),
                    op0=Alu.is_equal, op1=Alu.mult,
                )
                nc.vector.tensor_add(cw[:], cw[:], cwb[:])
                # fold in scale[e]/64 (the /64 matches the x64 Bm upload)
                nc.vector.tensor_tensor(cw[:], cw[:], sc_sb[:], Alu.mult)
                hw = hbuf.tile([P, ER], F16, tag="hw")
                nc.vector.tensor_tensor(
                    hw.rearrange("p (e r) -> p e r", e=E),
                    ps_h.rearrange("p (e r) -> p e r", e=E),
                    cw[:, :, None].to_broadcast([P, E, R]),
                    Alu.mult,
                )
                return hw

            def emit_router_pe(t, hw):
                """PE transposes of weighted h + fp8 cast on copy-back."""
                ps_base, ps_h, _ = pend[t]
                ps_t = ps_t_pool.tile([P, ER], F16, tag="t")
                for j in range(4):
                    nc.tensor.transpose(
                        ps_t[:, ts(j, P)], hw[:, ts(j, P)], ident[:]
                    )
                hwT = hbuf.tile([P, 4, P], F8, tag="hwT")
                nc.vector.tensor_copy(hwT.rearrange("p a b -> p (a b)"), ps_t[:])
                pend[t] = (ps_base, ps_h, hwT)

            def emit_B_and_out(t):
                """LoRA phase B (fp8 DoubleRow) into base psum, bias, store."""
                ps_base, _, hwT = pend.pop(t)
                for j in range(2):
                    nc.tensor.matmul(
                        ps_base[:, 0:512], hwT[:, 2 * j:2 * j + 2, :],
                        bm_sb[:, 2 * j:2 * j + 2, 0:512],
                        start=False, stop=(j == 1),
                        perf_mode=DR, skip_group_check=True,
                    )
                    nc.tensor.matmul(
                        ps_base[:, 512:1024], hwT[:, 2 * j:2 * j + 2, :],
                        bm_sb[:, 2 * j:2 * j + 2, 512:1024],
                        start=False, stop=(j == 1),
                        perf_mode=DR, skip_group_check=True,
                    )
                out_sb = obuf.tile([P, D], F32, tag="out")
                nc.vector.scalar_tensor_tensor(
                    out_sb[:], ps_base[:], 1.0 / 64.0, b2b_sb[:],
                    op0=Alu.mult, op1=Alu.add,
                )
                nc.scalar.dma_start(y_d[ts(t, P), :], out_sb[:])

            # ---- startup: phase A of tiles 0,1 fills the PE while wcat16
            # streams in; then base groups interleave tiles 0/1 ----
            alloc_psums(0)
            alloc_psums(1)
            emit_A(0)
            emit_base_group(0, 0)
            emit_A(1)
            for g in range(1, WG_K_GROUPS):
                emit_base_group(0, g)
                emit_base_group(1, g - 1)
            emit_xlo(0)
            hw0 = emit_router_dve(0)
            emit_base_group(1, WG_K_GROUPS - 1)
            emit_router_pe(0, hw0)
            emit_xlo(1)
            hw_pend = {1: emit_router_dve(1)}
            emit_B_and_out(0)

            # ---- steady state ----
            for t in range(2, TT - 1):
                if t >= 4:
                    alloc_x(t)
                    dma_x(t)
                alloc_psums(t)
                emit_A(t)
                if (t - 1) in hw_pend:
                    emit_router_pe(t - 1, hw_pend.pop(t - 1))
                for g in range(WG_K_GROUPS):
                    emit_base_group(t, g)
                    if g == 4:
                        emit_B_and_out(t - 1)
                emit_xlo(t)
                hw_pend[t] = emit_router_dve(t)

            # ---- last tile: phase A + router columns first so the DVE
            # router chain overlaps the base k-loop; transposes injected
            # mid-loop; phase B tail with split evac ----
            t = TT - 1
            alloc_x(t)
            dma_x(t)
            alloc_psums(t)
            xtb_sb, _, _ = xts[t]
            ps_base, ps_h, _ = pend[t]
            ps_l = lhalf(t)
            emit_A(t)
            emit_router_pe(t - 1, hw_pend.pop(t - 1))
            for k in range(KCH):
                nc.tensor.matmul(
                    ps_l[:, 0:16], xtb_sb[:, k, :], wc(k, D, NW16),
                    start=False, stop=(k == KCH - 1), skip_group_check=True,
                )
            emit_xlo(t)
            hw_last = emit_router_dve(t)
            for k in range(KCH):
                st = k == 0
                nc.tensor.matmul(
                    ps_base[:, 0:512], xtb_sb[:, k, :], wc(k, 0, 512),
                    start=st, stop=False, skip_group_check=True,
                )
                nc.tensor.matmul(
                    ps_base[:, 512:1024], xtb_sb[:, k, :], wc(k, 512, 1024),
                    start=st, stop=False, skip_group_check=True,
                )
                if k == 4:
                    emit_B_and_out(t - 1)
                if k == 10:
                    emit_router_pe(t, hw_last)
            # drain: last tile's phase B with split evac so the first half's
            # bias-add + store overlap the second half's matmuls
            ps_base, _, hwT = pend.pop(t)
            for j in range(2):
                nc.tensor.matmul(
                    ps_base[:, 0:512], hwT[:, 2 * j:2 * j + 2, :],
                    bm_sb[:, 2 * j:2 * j + 2, 0:512],
                    start=False, stop=(j == 1),
                    perf_mode=DR, skip_group_check=True,
                )
            out_sb = obuf.tile([P, D], F32, tag="out")
            nc.vector.scalar_tensor_tensor(
                out_sb[:, 0:512], ps_base[:, 0:512], 1.0 / 64.0,
                b2b_sb[:, 0:512], op0=Alu.mult, op1=Alu.add,
            )
            nc.scalar.dma_start(y_d[ts(t, P), 0:512], out_sb[:, 0:512])
            for j in range(2):
                nc.tensor.matmul(
                    ps_base[:, 512:1024], hwT[:, 2 * j:2 * j + 2, :],
                    bm_sb[:, 2 * j:2 * j + 2, 512:1024],
                    start=False, stop=(j == 1),
                    perf_mode=DR, skip_group_check=True,
                )
            nc.vector.scalar_tensor_tensor(
                out_sb[:, 512:1024], ps_base[:, 512:1024], 1.0 / 64.0,
                b2b_sb[:, 512:1024], op0=Alu.mult, op1=Alu.add,
            )
            nc.scalar.dma_start(y_d[ts(t, P), 512:1024], out_sb[:, 512:1024])

    nc.finalize()
    return nc


F8NP = ml_dtypes.float8_e4m3


def _prep_shared(Wg, W2, b2, A, Bm, scale):
    """Host-side weight layout prep (replicated across cores)."""
    f16, f32 = np.float16, np.float32
    # wcat16 = [W2*64 | Wg_hi | Wg_lo], k-chunked to [128, 32, NW16]
    wg_hi = Wg.astype(f16)
    wg_lo = (Wg.astype(f32) - wg_hi.astype(f32)).astype(f16)
    wcat = np.empty((H, NW16), dtype=f16)
    wcat[:, 0:D] = (W2.astype(f32) * 64.0).astype(f16)
    wcat[:, D:D + 8] = wg_hi
    wcat[:, D + 8:] = wg_lo
    wcat = np.ascontiguousarray(wcat.reshape(KCH, P, NW16).transpose(1, 0, 2))

    # acat8 = [A_flat*64 | Wg*64], k-chunked to [128, 32, NW8] fp8
    a_flat = np.ascontiguousarray(A.transpose(1, 0, 2)).reshape(H, ER)
    acat = np.empty((H, NW8), dtype=F8NP)
    acat[:, 0:ER] = (a_flat.astype(f32) * 64.0).astype(F8NP)
    acat[:, ER:] = (Wg.astype(f32) * 64.0).astype(F8NP)
    acat = np.ascontiguousarray(acat.reshape(KCH, P, NW8).transpose(1, 0, 2))

    # Bm*64 (scale NOT folded; it rides in the DVE combine weights),
    # [(e r), d] -> [128, 4, D] fp8
    bms = (Bm.astype(f32) * 64.0).reshape(ER, D)
    bms = np.ascontiguousarray(
        bms.reshape(4, P, D).transpose(1, 0, 2)
    ).astype(F8NP)

    b2b = np.ascontiguousarray(
        np.broadcast_to(b2.astype(f32)[None, :], (P, D))
    )
    scb = np.ascontiguousarray(
        np.broadcast_to((scale.astype(f32) / 64.0)[None, :], (P, E))
    )
    return wcat, acat, bms, b2b, scb


def _prep_x_core(x_c):
    """Per-core x prep: fp16 + fp8 copies + scaled-fp8 lo residual,
    [tile, p, k, ti] layout."""
    f16, f32 = np.float16, np.float32
    xtb = x_c.astype(f16)                                   # [1024, 4096]
    x8 = x_c.astype(F8NP)
    xlo = ((x_c.astype(f32) - xtb.astype(f32)) * 2048.0).astype(F8NP)

    def lay(a):
        return np.ascontiguousarray(
            a.reshape(TT, P, KCH, P).transpose(0, 3, 2, 1)
        )
    return lay(xtb), lay(x8), lay(xlo)


def kernel(x, Wg, W2, b2, A, Bm, scale):
    x = np.asarray(x, dtype=np.float32)
    Wg = np.asarray(Wg, dtype=np.float32)
    W2 = np.asarray(W2, dtype=np.float32)
    b2 = np.asarray(b2, dtype=np.float32)
    A = np.asarray(A, dtype=np.float32)
    Bm = np.asarray(Bm, dtype=np.float32)
    scale = np.asarray(scale, dtype=np.float32)

    if "nc" not in _CACHE:
        _CACHE["nc"] = _build_nc()
    nc = _CACHE["nc"]

    wcat, acat, bms, b2b, scb = _prep_shared(Wg, W2, b2, A, Bm, scale)
    in_maps = []
    for c in range(NCORES):
        xtb, x8, xlo = _prep_x_core(x[c])
        in_maps.append({
            "xtb": xtb, "x8": x8, "xlo": xlo, "wcat": wcat,
            "acat": acat, "bm": bms, "b2b": b2b, "sc": scb,
        })

    res = run_bass_kernel_spmd(nc, in_maps, core_ids=list(range(NCORES)))
    out = np.stack([res.results[c]["y"] for c in range(NCORES)], axis=0)
    return out.astype(np.float32)
